# revision 30
# baseline (speedup 1.0000x reference)
"""Trainium2 Bass kernel for GQA attention (B=2, S=1024, HID=4096, H=32,
HKV=8, HD=128) with NeoX rotary + additive mask, sharded over 8 NeuronCores.

Sharding: 2 data-parallel groups (one per batch sequence) x 4-way tensor
parallel (8 q-heads / 2 kv-heads per core). wq/wk/wv column-sharded.
After attention, the per-core head outputs (bf16, [1024 feat, tok]) are
AllGathered within each 4-core group (2 x 1MB AG per group, fired per
512-token half so the first overlaps the second half's attention), then each
core computes the output projection over the FULL 4096-feature contraction
for its quarter of the HID columns (wo column-sharded). No ReduceScatter and
no fp32 partial-sum traffic; the host concatenates the 8 disjoint
[1024 x 1024] output blocks.

Everything on device runs in a transposed layout ([feature, token]) so every
matmul streams with free-dim 512 at full PE rate (fp32r for attention scores,
bf16 inputs for QKV/PV/wo with fp32 PSUM accumulation).
"""

import math

import ml_dtypes
import numpy as np

B, S, HID, H, HKV, HD = 2, 1024, 4096, 32, 8, 128
NCORES = 8
TPG = 4                      # tensor-parallel group size
NGROUPS = NCORES // TPG      # data-parallel groups (= B)
HL = H // TPG                # q heads per core (8)
KVL = HKV // TPG             # kv heads per core (2)
GQ = H // HKV                # q heads per kv head (4)
SCALE = 1.0 / math.sqrt(HD)
QB = 512                     # q block (free dim of attention matmuls)
NHALF = S // QB              # token halves (2)
HIDL = HID // TPG            # output columns per core (1024)
NEG_THRESH = -1.0e8          # mask values <= this count as fully masked

_STATE: dict = {}


# ----------------------------------------------------------------------------
# walrus compat: this toolchain supports at most ONE semaphore wait per
# instruction; Tile's scheduler can attach several. Hoist extras onto
# same-engine nops placed immediately before the instruction.
# ----------------------------------------------------------------------------
def _split_multi_waits(nc):
    import concourse.mybir as mybir

    def detached_nop(engine_type):
        bi = nc.engines[engine_type].nop()
        inst = bi.ins
        for fn in nc.m.functions:
            for b in fn.blocks:
                il = b.instructions
                if il and il[-1].name == inst.name:
                    il.pop()
                    return inst
        raise AssertionError("could not detach nop")

    for fn in nc.m.functions:
        for b in fn.blocks:
            il = b.instructions
            out = []
            changed = False
            for inst in il:
                si = inst.sync_info
                waits = list(si.on_wait) if (si is not None and si.on_wait) else []
                if len(waits) > 1:
                    for w in waits[:-1]:
                        nop = detached_nop(inst.engine)
                        nop.sync_info = mybir.SyncInfo(on_wait=[w], on_update=[])
                        out.append(nop)
                    si.on_wait = waits[-1:]
                    changed = True
                out.append(inst)
            if changed:
                b.instructions = out


# ----------------------------------------------------------------------------
# Device program
# ----------------------------------------------------------------------------
def _build_module(mask_desc, mask_binary):
    """mask_desc: per (qb, kb) block descriptor list computed on the host from
    the actual attn_mask:
      ("skip",)                 block fully masked
      ("full", need_mask:bool)  full 512-wide block, optionally + mask data
      ("causal", off:int)       causal window: cols [off,512) active, mask
                                on the 128-wide diagonal window at `off`
    mask_binary: True when every mask value is 0 or -inf-like; masking is then
    a post-exp multiply by a 0/1 bf16 mask (off the scores critical path).
    Otherwise the mask is added to the scores in PSUM before exp (exact for
    arbitrary additive masks).
    """
    import concourse.bass as bass
    import concourse.mybir as mybir
    import concourse.tile as tile
    from concourse.masks import make_identity

    dt = mybir.dt
    f32, f32r, bf16 = dt.float32, dt.float32r, dt.bfloat16
    KT = HID // 128  # 32 contraction tiles
    NXT = 8          # x chunks (4 kt tiles each)
    XKT = KT // NXT

    nc = bass.Bass()

    # --- DRAM parameters (per-core shards, host-prepared) ---
    xt_in = nc.declare_dram_parameter("xt", [KT, 128, S], bf16, isOutput=False)
    wq_in = nc.declare_dram_parameter("wq", [HL, 128, KT, 128], bf16, isOutput=False)
    wk_in = nc.declare_dram_parameter("wk", [KVL, 128, KT, 128], bf16, isOutput=False)
    wv_in = nc.declare_dram_parameter("wv", [KVL, 128, KT, 128], bf16, isOutput=False)
    wo_in = nc.declare_dram_parameter("wo", [KT, 128, HIDL], bf16, isOutput=False)
    cos_in = nc.declare_dram_parameter("cos_t", [128, S], f32, isOutput=False)
    sin_in = nc.declare_dram_parameter("sin_t", [128, S], f32, isOutput=False)
    # mask blocks actually referenced by the program, in transposed [kv, q]
    # layout; index map built below.
    mask_tiles = []
    for qb in range(NHALF):
        for kb in range(S // 128):
            d = mask_desc[qb][kb]
            if d[0] == "causal":
                mask_tiles.append((qb, kb, 128))
            elif d[0] == "full" and d[1]:
                mask_tiles.append((qb, kb, QB))
    nmask = max(1, len(mask_tiles))
    mw = max([t[2] for t in mask_tiles], default=128)
    mdt = bf16 if mask_binary else f32
    mask_in = nc.declare_dram_parameter("maskt", [nmask, 128, mw], mdt, isOutput=False)
    out_ext = nc.declare_dram_parameter("outp", [S, HIDL], f32, isOutput=True)

    from contextlib import ExitStack
    ctx = ExitStack()
    with tile.TileContext(nc) as tc:
        const = ctx.enter_context(tc.tile_pool(name="const", bufs=1))
        dram = ctx.enter_context(tc.tile_pool(name="dram", bufs=1, space="DRAM"))
        qkvpool = ctx.enter_context(tc.tile_pool(name="qkv", bufs=1))

        # AG buffers: per token half, per-core heads in [feat, tok] layout.
        ag_in = [dram.tile([HL * HD, QB], bf16, tag=f"agi{i}", name=f"ag_in{i}")
                 for i in range(NHALF)]
        ag_out = [dram.tile([TPG * HL * HD, QB], bf16, tag=f"ago{i}",
                            name=f"ag_out{i}")
                  for i in range(NHALF)]

        cos_t = const.tile([128, S], f32, tag="cos")
        sin_t = const.tile([128, S], f32, tag="sin")
        ones32 = const.tile([128, 128], f32, tag="ones32")
        nc.gpsimd.memset(ones32[:], 1.0)
        ones_t = const.tile([128, 128], bf16, tag="ones")
        nc.vector.tensor_copy(ones_t[:], ones32[:])
        ident = const.tile([128, 128], f32, tag="ident")
        make_identity(nc, ident[:])
        mask_sb = const.tile([128, nmask, mw], mdt, tag="mask")
        mask_idx = {(qb, kb): i for i, (qb, kb, _) in enumerate(mask_tiles)}

        # activations that live through phase 2 (freed before phase 3)
        q_rot = [qkvpool.tile([128, S], f32r, tag=f"q{h}", name=f"q_rot{h}")
                 for h in range(HL)]
        k_rot = [qkvpool.tile([128, S], f32r, tag=f"k{j}", name=f"k_rot{j}")
                 for j in range(KVL)]
        v_nat = [qkvpool.tile([128, S // 128, 128], bf16, tag=f"v{j}", name=f"v_nat{j}")
                 for j in range(KVL)]

        # ---------------- phase 1: QKV projections + rotary -----------------
        with tc.tile_pool(name="p1x", bufs=1) as xpool, \
             tc.tile_pool(name="p1w", bufs=4) as wpool, \
             tc.tile_pool(name="p1t", bufs=2) as tpool, \
             tc.tile_pool(name="p1ps", bufs=4, space="PSUM") as pspool, \
             tc.tile_pool(name="p1pst", bufs=2, space="PSUM") as pstr:

            # DMA issue order matters: the first matmuls need wk ct0 and the
            # leading x chunks, so interleave the k/v weight loads with
            # fine-grained x chunks instead of monolithic 2MB x DMAs.
            xt = [xpool.tile([128, XKT, S], bf16, tag=f"xt{i}", name=f"xt{i}")
                  for i in range(NXT)]
            w_kv = [wpool.tile([128, KT, 128], bf16, tag="w", name=f"wkv{j}")
                    for j in range(2 * KVL)]
            nc.sync.dma_start(out=w_kv[0][:], in_=wk_in[0])
            for i in range(NXT):
                nc.sync.dma_start(
                    out=xt[i][:],
                    in_=xt_in[i * XKT:(i + 1) * XKT, :, :]
                        .rearrange("k p t -> p k t"),
                )
                if i == 0:
                    nc.sync.dma_start(out=w_kv[1][:], in_=wk_in[1])
                elif i == 1:
                    nc.sync.dma_start(out=cos_t[:], in_=cos_in[:])
                    nc.sync.dma_start(out=sin_t[:], in_=sin_in[:])
                    nc.sync.dma_start(
                        out=mask_sb[:],
                        in_=mask_in[:].rearrange("b p c -> p b c"))
                elif i == 2:
                    nc.sync.dma_start(out=w_kv[2][:], in_=wv_in[0])
                elif i == 3:
                    nc.sync.dma_start(out=w_kv[3][:], in_=wv_in[1])

            def xt_sl(kt, tb):
                return xt[kt // XKT][:, kt % XKT, tb * QB:(tb + 1) * QB]

            # (dram tensor, n col tiles, kind, preloaded tiles or None)
            projs = [(wk_in, KVL, "k", w_kv[0:KVL]), (wv_in, KVL, "v", w_kv[KVL:]),
                     (wq_in, HL, "q", None)]
            for w_dram, ncts, kind, pre in projs:
                for ct in range(ncts):
                    if pre is not None:
                        w_sb = pre[ct]
                    else:
                        w_sb = wpool.tile([128, KT, 128], bf16, tag="w")
                        nc.sync.dma_start(out=w_sb[:], in_=w_dram[ct])
                    for tb in range(S // QB):
                        ps = pspool.tile([128, QB], f32, tag="ps_qkv")
                        for kt in range(KT):
                            nc.tensor.matmul(
                                ps[:],
                                w_sb[:, kt, :],
                                xt_sl(kt, tb),
                                start=(kt == 0),
                                stop=(kt == KT - 1),
                            )
                        tsl = slice(tb * QB, (tb + 1) * QB)
                        if kind in ("q", "k"):
                            dest = q_rot[ct] if kind == "q" else k_rot[ct]
                            swap = tpool.tile([128, QB], f32, tag="swap")
                            nc.scalar.activation(
                                swap[0:64, :], ps[64:128, :],
                                mybir.ActivationFunctionType.Copy, scale=-1.0)
                            nc.scalar.activation(
                                swap[64:128, :], ps[0:64, :],
                                mybir.ActivationFunctionType.Copy)
                            t2 = tpool.tile([128, QB], f32, tag="t2")
                            nc.vector.tensor_tensor(
                                t2[:], ps[:], cos_t[:, tsl], mybir.AluOpType.mult)
                            t3 = tpool.tile([128, QB], f32, tag="t3")
                            nc.vector.tensor_tensor(
                                t3[:], swap[:], sin_t[:, tsl], mybir.AluOpType.mult)
                            nc.vector.tensor_tensor(
                                dest[:, tsl], t2[:], t3[:], mybir.AluOpType.add)
                        else:  # v: transpose to natural [t, d] layout
                            vt = tpool.tile([128, QB], f32, tag="vt")
                            nc.scalar.activation(
                                vt[:], ps[:], mybir.ActivationFunctionType.Copy)
                            for j in range(QB // 128):
                                ps_t = pstr.tile([128, 128], f32, tag="ps_tr")
                                nc.tensor.transpose(
                                    ps_t[:], vt[:, j * 128:(j + 1) * 128], ident[:])
                                nc.vector.tensor_copy(
                                    v_nat[ct][:, tb * (QB // 128) + j, :], ps_t[:])

        # ---------------- phase 2: attention, AG fired per token half -------
        # wo weights prefetched during attention (resident bf16
        # [p(feat within ct), ct, hid-quarter]).
        wopool = ctx.enter_context(tc.tile_pool(name="wo", bufs=1))
        wo_sb = wopool.tile([128, KT, HIDL], bf16, tag="wo")
        nc.sync.dma_start(
            out=wo_sb[:], in_=wo_in[:, :, :].rearrange("c p n -> p c n"))

        with tc.tile_pool(name="p2p", bufs=4) as ppool, \
             tc.tile_pool(name="p2r", bufs=2) as rpool, \
             tc.tile_pool(name="p2o", bufs=4) as aopool, \
             tc.tile_pool(name="p3a", bufs=3) as agpool, \
             tc.tile_pool(name="p3o", bufs=4) as opool:

            def ag_chunk_dma(half, tl, eng=None):
                t = agpool.tile([128, KT, 128], bf16, tag="ag",
                                name=f"ag_sb{half}_{tl}")
                # The first chunks go out on the (otherwise idle) gpsimd
                # engine: the sync/ACT queues are full of attention work that
                # becomes ready later, and in-order dispatch there would stall
                # the wo start behind it.
                (eng or nc.scalar).dma_start(
                    out=t[:],
                    in_=ag_out[half][:, tl * 128:(tl + 1) * 128]
                        .rearrange("(c p) t -> p c t", p=128))
                return t

            prefetched = {}

            attnps = ExitStack()
            scpool = attnps.enter_context(
                tc.tile_pool(name="p2sc", bufs=2, space="PSUM"))
            pvpool = attnps.enter_context(
                tc.tile_pool(name="p2pv", bufs=2, space="PSUM"))
            dnpool = attnps.enter_context(
                tc.tile_pool(name="p2dn", bufs=2, space="PSUM"))
            pso = attnps.enter_context(
                tc.tile_pool(name="p3ps", bufs=2, space="PSUM"))

            # token half 1 first: its attention is PE-dense and its AG then
            # overlaps half 0's DVE/ACT-heavy attention, so the wo phase can
            # start the moment attention ends.
            for qb in reversed(range(NHALF)):
                for h in range(HL):
                    kvh = h // GQ
                    blocks = []  # (kb, off, need_mask)
                    for kb in range(S // 128):
                        d = mask_desc[qb][kb]
                        if d[0] == "skip":
                            continue
                        if d[0] == "causal":
                            blocks.append((kb, d[1], ("diag", d[1])))
                        else:
                            blocks.append((kb, 0, ("full",) if d[1] else None))
                    ps_pv = pvpool.tile([128, QB], f32, tag="ps_pv")
                    ps_dn = dnpool.tile([128, QB], f32, tag="ps_dn")
                    nblk = len(blocks)
                    for bi, (kb, off, mk) in enumerate(blocks):
                        qsl = slice(qb * QB + off, (qb + 1) * QB)
                        ps_sc = scpool.tile([128, QB], f32, tag="ps_sc")
                        nc.tensor.matmul(
                            ps_sc[:, off:QB],
                            k_rot[kvh][:, kb * 128:(kb + 1) * 128],
                            q_rot[h][:, qsl],
                            start=True, stop=True,
                        )
                        if mk is not None and not mask_binary:
                            mi = mask_idx[(qb, kb)]
                            if mk[0] == "diag":
                                nc.vector.tensor_tensor(
                                    ps_sc[:, off:off + 128], ps_sc[:, off:off + 128],
                                    mask_sb[:, mi, 0:128], mybir.AluOpType.add)
                            else:
                                nc.vector.tensor_tensor(
                                    ps_sc[:, 0:QB], ps_sc[:, 0:QB],
                                    mask_sb[:, mi, 0:QB], mybir.AluOpType.add)
                        p_t = ppool.tile([128, QB], bf16, tag="p")
                        nc.scalar.activation(
                            p_t[:, off:QB], ps_sc[:, off:QB],
                            mybir.ActivationFunctionType.Exp)
                        if mk is not None and mask_binary:
                            # exp(s+m) == exp(s) * [m == 0]: zero the masked
                            # entries after exp, off the scores->exp chain
                            mi = mask_idx[(qb, kb)]
                            if mk[0] == "diag":
                                nc.vector.tensor_tensor(
                                    p_t[:, off:off + 128], p_t[:, off:off + 128],
                                    mask_sb[:, mi, 0:128], mybir.AluOpType.mult)
                            else:
                                nc.vector.tensor_tensor(
                                    p_t[:, 0:QB], p_t[:, 0:QB],
                                    mask_sb[:, mi, 0:QB], mybir.AluOpType.mult)
                        nc.tensor.matmul(
                            ps_pv[:, off:QB],
                            v_nat[kvh][:, kb, :],
                            p_t[:, off:QB],
                            start=(bi == 0), stop=(bi == nblk - 1),
                        )
                        nc.tensor.matmul(
                            ps_dn[:, off:QB],
                            ones_t[:],
                            p_t[:, off:QB],
                            start=(bi == 0), stop=(bi == nblk - 1),
                        )
                    recip = rpool.tile([128, QB], f32, tag="recip")
                    nc.vector.reciprocal(recip[:], ps_dn[:])
                    o_attn = aopool.tile([128, QB], bf16, tag="oat")
                    nc.vector.tensor_tensor(
                        o_attn[:], ps_pv[:], recip[:], mybir.AluOpType.mult)
                    nc.sync.dma_start(
                        out=ag_in[qb][h * HD:(h + 1) * HD, :], in_=o_attn[:])

                # all heads of this token half written: gather within group
                nc.gpsimd.collective_compute(
                    "AllGather", mybir.AluOpType.bypass,
                    replica_groups=[list(range(g * TPG, (g + 1) * TPG))
                                    for g in range(NGROUPS)],
                    ins=[ag_in[qb].opt()], outs=[ag_out[qb].opt()],
                )
                if qb == NHALF - 1:
                    # prefetch the first wo chunks right behind this AG, ahead
                    # of half 0's output DMAs in the queue (avoids
                    # head-of-line blocking on the wo start).
                    for tl in range(2):
                        prefetched[(qb, tl)] = ag_chunk_dma(qb, tl, nc.gpsimd)

            # ------------- phase 3: output projection -----------------------
            NHB = HIDL // QB  # 2 hid blocks of 512
            for half in reversed(range(NHALF)):
                for tl in range(QB // 128):       # token tiles within half
                    tt = half * (QB // 128) + tl
                    ag_sb = prefetched.pop((half, tl), None)
                    if ag_sb is None:
                        ag_sb = ag_chunk_dma(half, tl)
                    for hb in range(NHB):
                        ps_o = pso.tile([128, QB], f32, tag="ps_o")
                        for ct in range(KT):
                            nc.tensor.matmul(
                                ps_o[:],
                                ag_sb[:, ct, :],
                                wo_sb[:, ct, hb * QB:(hb + 1) * QB],
                                start=(ct == 0), stop=(ct == KT - 1),
                            )
                        o_sb = opool.tile([128, QB], f32, tag="o")
                        if (hb + tt) % 2 == 0:
                            nc.vector.tensor_copy(o_sb[:], ps_o[:])
                        else:
                            nc.scalar.activation(
                                o_sb[:], ps_o[:],
                                mybir.ActivationFunctionType.Copy)
                        nc.sync.dma_start(
                            out=out_ext[tt * 128:(tt + 1) * 128,
                                        hb * QB:(hb + 1) * QB],
                            in_=o_sb[:])
            attnps.close()

        ctx.close()

    _split_multi_waits(nc)
    return nc, [t[:2] for t in mask_tiles], mw


# ----------------------------------------------------------------------------
# Host-side input prep
# ----------------------------------------------------------------------------
def _classify_mask(attn_mask):
    """Per (qb, kb) descriptor from the actual mask contents (transposed
    [kv, q] view). Causal masks produce the efficient windowed structure."""
    mt = attn_mask.T  # [kv, q]
    desc = []
    for qb in range(S // QB):
        row = []
        q0 = qb * QB
        for kb in range(S // 128):
            blk = mt[kb * 128:(kb + 1) * 128, q0:q0 + QB]
            if np.all(blk <= NEG_THRESH):
                row.append(("skip",))
                continue
            if np.all(np.abs(blk) < 1e-6):
                row.append(("full", False))
                continue
            # causal window? cols [0, off) fully masked, diag at [off, off+128),
            # cols beyond fully visible
            off = kb * 128 - q0
            causal = False
            if 0 <= off <= QB - 128:
                left_ok = np.all(blk[:, :off] <= NEG_THRESH) if off else True
                right_ok = (np.all(np.abs(blk[:, off + 128:]) < 1e-6)
                            if off + 128 < QB else True)
                causal = bool(left_ok and right_ok)
            if causal:
                row.append(("causal", off))
            else:
                row.append(("full", True))
        desc.append(row)
    # every q column must keep at least one contributing block
    for qb in range(S // QB):
        assert any(d[0] != "skip" for d in desc[qb]), "fully-masked q rows unsupported"
    return desc


def _mask_is_binary(attn_mask):
    """True when every mask entry is (near) 0 or fully-masked, so masking can
    run as a post-exp multiply by 0/1."""
    return bool(np.all((np.abs(attn_mask) < 1e-6) | (attn_mask <= NEG_THRESH)))


def _prep_core_inputs(inputs, mask_desc, mask_list, mw, mask_binary):
    x = np.asarray(inputs["x"], np.float32)
    wq = np.asarray(inputs["wq"], np.float32)
    wk = np.asarray(inputs["wk"], np.float32)
    wv = np.asarray(inputs["wv"], np.float32)
    wo = np.asarray(inputs["wo"], np.float32)
    attn_mask = np.asarray(inputs["attn_mask"], np.float32)
    start_pos = np.asarray(inputs["start_pos"], np.int32)

    bf = ml_dtypes.bfloat16
    KT = HID // 128

    inv_freq = 1.0 / (10000.0 ** (np.arange(0, HD, 2, dtype=np.float32) / HD))
    mt = attn_mask.T
    if mask_list:
        mask_arr = np.zeros((len(mask_list), 128, mw), np.float32)
        for i, (qb, kb) in enumerate(mask_list):
            d = mask_desc[qb][kb]
            if d[0] == "causal":
                off = d[1]
                mask_arr[i, :, 0:128] = mt[kb * 128:(kb + 1) * 128,
                                           qb * QB + off:qb * QB + off + 128]
            else:
                mask_arr[i, :, 0:QB] = mt[kb * 128:(kb + 1) * 128,
                                          qb * QB:(qb + 1) * QB]
    else:
        mask_arr = np.zeros((1, 128, mw), np.float32)
    if mask_binary:
        mask_arr = (mask_arr > NEG_THRESH).astype(ml_dtypes.bfloat16)

    # lhsT tile layout: [ct, p=hid_within_kt, kt, col_within_ct]
    def wtile2(w):
        c = w.shape[1]
        return np.ascontiguousarray(
            w.reshape(KT, 128, c // 128, 128).transpose(2, 1, 0, 3))

    in_maps = []
    for core in range(NCORES):
        g, r = divmod(core, TPG)
        xb = x[g * S:(g + 1) * S]                       # [S, HID]
        xt = np.ascontiguousarray(xb.T.reshape(KT, 128, S)).astype(bf)
        wq_c = (wq[:, r * HL * HD:(r + 1) * HL * HD] * SCALE)
        wk_c = wk[:, r * KVL * HD:(r + 1) * KVL * HD]
        wv_c = wv[:, r * KVL * HD:(r + 1) * KVL * HD]
        wo_c = wo[:, r * HIDL:(r + 1) * HIDL]           # [4096, 1024]

        pos = start_pos[g] + np.arange(S, dtype=np.float32)
        ang = pos[:, None] * inv_freq[None, :]          # [S, HD/2]
        cos = np.concatenate([np.cos(ang), np.cos(ang)], -1).T  # [HD, S]
        sin = np.concatenate([np.sin(ang), np.sin(ang)], -1).T

        in_maps.append({
            "xt": xt,
            "wq": wtile2(wq_c).astype(bf),
            "wk": wtile2(wk_c).astype(bf),
            "wv": wtile2(wv_c).astype(bf),
            "wo": np.ascontiguousarray(wo_c.reshape(KT, 128, HIDL)).astype(bf),
            "cos_t": np.ascontiguousarray(cos.astype(np.float32)),
            "sin_t": np.ascontiguousarray(sin.astype(np.float32)),
            "maskt": mask_arr,
        })
    return in_maps


def _make_runner(nc):
    """Cached jit over the bass module (adapted from
    concourse.bass2jax.run_bass_via_pjrt so repeat calls reuse one NEFF)."""
    import jax
    import jax.numpy as jnp
    from jax.sharding import Mesh, NamedSharding, PartitionSpec
    from jax.experimental.shard_map import shard_map

    import concourse.mybir as mybir
    from concourse import bass2jax

    bass2jax.install_neuronx_cc_hook()
    assert nc.dbg_addr is None
    partition_name = (nc.partition_id_tensor.name
                      if nc.partition_id_tensor else None)

    in_names, out_names, out_avals, out_shapes = [], [], [], []
    for alloc in nc.m.functions[0].allocations:
        if not isinstance(alloc, mybir.MemoryLocationSet):
            continue
        name = alloc.memorylocations[0].name
        if alloc.kind == "ExternalInput":
            if name != partition_name:
                in_names.append(name)
        elif alloc.kind == "ExternalOutput":
            assert alloc.tensor_shape is not None and alloc.dtype is not None
            shape = tuple(alloc.tensor_shape)
            npdt = mybir.dt.np(alloc.dtype)
            out_names.append(name)
            out_shapes.append((shape, npdt))
            out_avals.append(jax.core.ShapedArray(shape, npdt))

    n_params = len(in_names)
    n_outs = len(out_names)
    all_in_names = in_names + out_names
    if partition_name is not None:
        all_in_names = all_in_names + [partition_name]
    donate = tuple(range(n_params, n_params + n_outs))

    def _body(*args):
        operands = list(args)
        if partition_name is not None:
            operands.append(bass2jax.partition_id_tensor())
        outs = bass2jax._bass_exec_p.bind(
            *operands,
            out_avals=tuple(out_avals),
            in_names=tuple(all_in_names),
            out_names=tuple(out_names),
            lowering_input_output_aliases=(),
            sim_require_finite=True,
            sim_require_nnan=True,
            nc=nc,
        )
        return tuple(outs)

    devices = jax.devices()[:NCORES]
    mesh = Mesh(np.asarray(devices), ("core",))
    pc = PartitionSpec("core")
    sharded = jax.jit(
        shard_map(_body, mesh=mesh, in_specs=(pc,) * (n_params + n_outs),
                  out_specs=(pc,) * n_outs, check_rep=False),
        donate_argnums=donate, keep_unused=True)

    shard_dev = NamedSharding(mesh, pc)

    def make_zeros():
        return tuple(
            jax.device_put(np.zeros((NCORES * s[0], *s[1:]), d), shard_dev)
            for s, d in out_shapes)

    def put_inputs(in_maps):
        return [
            jax.device_put(
                np.concatenate([np.asarray(m[nm]) for m in in_maps], axis=0),
                shard_dev)
            for nm in in_names]

    def run_from_dev(in_dev, zeros):
        out_arrs = sharded(*in_dev, *zeros)
        jax.block_until_ready(out_arrs)
        return out_arrs

    def run(in_maps):
        out_arrs = run_from_dev(put_inputs(in_maps), make_zeros())
        return [
            {nm: np.asarray(out_arrs[i]).reshape(NCORES, *out_shapes[i][0])[c]
             for i, nm in enumerate(out_names)}
            for c in range(NCORES)]

    return {"run": run, "put_inputs": put_inputs, "make_zeros": make_zeros,
            "run_from_dev": run_from_dev, "sharded": sharded}


def _get_runner(mask_desc, mask_binary):
    key = (repr(mask_desc), mask_binary)
    if _STATE.get("key") == key:
        return _STATE["run"], _STATE["mask_list"], _STATE["mw"]

    nc, mask_list, mw = _build_module(mask_desc, mask_binary)
    runner = _make_runner(nc)

    _STATE.update({"key": key, "run": runner["run"], "mask_list": mask_list,
                   "mw": mw, "nc": nc, "runner": runner})
    return runner["run"], mask_list, mw


def kernel(**inputs) -> np.ndarray:
    attn_mask = np.asarray(inputs["attn_mask"], np.float32)
    mask_desc = _classify_mask(attn_mask)
    mask_binary = _mask_is_binary(attn_mask)
    run, mask_list, mw = _get_runner(mask_desc, mask_binary)
    in_maps = _prep_core_inputs(inputs, mask_desc, mask_list, mw, mask_binary)
    results = run(in_maps)
    out = np.empty((B * S, HID), np.float32)
    for core in range(NCORES):
        g, r = divmod(core, TPG)
        out[g * S:(g + 1) * S, r * HIDL:(r + 1) * HIDL] = results[core]["outp"]
    return out


# revision 32
# speedup vs baseline: 1.5878x; 1.5878x over previous
"""Trainium2 Bass kernel for GQA attention (B=2, S=1024, HID=4096, H=32,
HKV=8, HD=128) with NeoX rotary + additive mask, sharded over 8 NeuronCores.

Sharding: 2 data-parallel groups (one per batch sequence) x 4-way tensor
parallel (8 q-heads / 2 kv-heads per core). wq/wk/wv column-sharded.
After attention, the per-core head outputs (bf16, [1024 feat, tok]) are
AllGathered within each 4-core group (2 x 1MB AG per group, fired per
512-token half so the first overlaps the second half's attention), then each
core computes the output projection over the FULL 4096-feature contraction
for its quarter of the HID columns (wo column-sharded). No ReduceScatter and
no fp32 partial-sum traffic; the host concatenates the 8 disjoint
[1024 x 1024] output blocks.

Everything on device runs in a transposed layout ([feature, token]) so every
matmul streams with free-dim 512 at full PE rate (fp32r for attention scores,
bf16 inputs for QKV/PV/wo with fp32 PSUM accumulation).

`repeat` builds a module that runs the whole forward pass N times back to
back; timing R>1 against R=1 isolates true device execution time from the
per-dispatch host overhead (~350us on this axon-tunneled setup).
"""

import math

import ml_dtypes
import numpy as np

B, S, HID, H, HKV, HD = 2, 1024, 4096, 32, 8, 128
NCORES = 8
TPG = 4                      # tensor-parallel group size
NGROUPS = NCORES // TPG      # data-parallel groups (= B)
HL = H // TPG                # q heads per core (8)
KVL = HKV // TPG             # kv heads per core (2)
GQ = H // HKV                # q heads per kv head (4)
SCALE = 1.0 / math.sqrt(HD)
QB = 512                     # q block (free dim of attention matmuls)
NHALF = S // QB              # token halves (2)
HIDL = HID // TPG            # output columns per core (1024)
NEG_THRESH = -1.0e8          # mask values <= this count as fully masked

_STATE: dict = {}


# ----------------------------------------------------------------------------
# walrus compat: this toolchain supports at most ONE semaphore wait per
# instruction; Tile's scheduler can attach several. Hoist extras onto
# same-engine nops placed immediately before the instruction.
# ----------------------------------------------------------------------------
def _split_multi_waits(nc):
    import concourse.mybir as mybir

    def detached_nop(engine_type):
        bi = nc.engines[engine_type].nop()
        inst = bi.ins
        for fn in nc.m.functions:
            for b in fn.blocks:
                il = b.instructions
                if il and il[-1].name == inst.name:
                    il.pop()
                    return inst
        raise AssertionError("could not detach nop")

    for fn in nc.m.functions:
        for b in fn.blocks:
            il = b.instructions
            out = []
            changed = False
            for inst in il:
                si = inst.sync_info
                waits = list(si.on_wait) if (si is not None and si.on_wait) else []
                if len(waits) > 1:
                    for w in waits[:-1]:
                        nop = detached_nop(inst.engine)
                        nop.sync_info = mybir.SyncInfo(on_wait=[w], on_update=[])
                        out.append(nop)
                    si.on_wait = waits[-1:]
                    changed = True
                out.append(inst)
            if changed:
                b.instructions = out


# ----------------------------------------------------------------------------
# Device program
# ----------------------------------------------------------------------------
def _build_module(mask_desc, mask_binary, repeat=1):
    """mask_desc: per (qb, kb) block descriptor list computed on the host from
    the actual attn_mask:
      ("skip",)                 block fully masked
      ("full", need_mask:bool)  full 512-wide block, optionally + mask data
      ("causal", off:int)       causal window: cols [off,512) active, mask
                                on the 128-wide diagonal window at `off`
    mask_binary: True when every mask value is 0 or -inf-like; masking is then
    a post-exp multiply by a 0/1 bf16 mask (off the scores critical path).
    Otherwise the mask is added to the scores in PSUM before exp (exact for
    arbitrary additive masks).
    """
    import concourse.bass as bass
    import concourse.mybir as mybir
    import concourse.tile as tile
    from concourse.masks import make_identity
    from contextlib import ExitStack

    dt = mybir.dt
    f32, f32r, bf16 = dt.float32, dt.float32r, dt.bfloat16
    KT = HID // 128  # 32 contraction tiles
    NXT = 8          # x chunks (4 kt tiles each)
    XKT = KT // NXT

    nc = bass.Bass()

    # --- DRAM parameters (per-core shards, host-prepared) ---
    xt_in = nc.declare_dram_parameter("xt", [KT, 128, S], bf16, isOutput=False)
    wq_in = nc.declare_dram_parameter("wq", [HL, 128, KT, 128], bf16, isOutput=False)
    wk_in = nc.declare_dram_parameter("wk", [KVL, 128, KT, 128], bf16, isOutput=False)
    wv_in = nc.declare_dram_parameter("wv", [KVL, 128, KT, 128], bf16, isOutput=False)
    wo_in = nc.declare_dram_parameter("wo", [KT, 128, HIDL], bf16, isOutput=False)
    cos_in = nc.declare_dram_parameter("cos_t", [128, S], f32, isOutput=False)
    sin_in = nc.declare_dram_parameter("sin_t", [128, S], f32, isOutput=False)
    # mask blocks actually referenced by the program, in transposed [kv, q]
    # layout; index map built below.
    mask_tiles = []
    for qb in range(NHALF):
        for kb in range(S // 128):
            d = mask_desc[qb][kb]
            if d[0] == "causal":
                mask_tiles.append((qb, kb, 128))
            elif d[0] == "full" and d[1]:
                mask_tiles.append((qb, kb, QB))
    nmask = max(1, len(mask_tiles))
    mw = max([t[2] for t in mask_tiles], default=128)
    mdt = bf16 if mask_binary else f32
    mask_in = nc.declare_dram_parameter("maskt", [nmask, 128, mw], mdt, isOutput=False)
    out_ext = nc.declare_dram_parameter("outp", [S, HIDL], f32, isOutput=True)

    ctx = ExitStack()
    with tile.TileContext(nc) as tc:
        const = ctx.enter_context(tc.tile_pool(name="const", bufs=1))
        dram = ctx.enter_context(tc.tile_pool(name="dram", bufs=1, space="DRAM"))

        cos_t = const.tile([128, S], f32, tag="cos")
        sin_t = const.tile([128, S], f32, tag="sin")
        ones32 = const.tile([128, 128], f32, tag="ones32")
        nc.gpsimd.memset(ones32[:], 1.0)
        ones_t = const.tile([128, 128], bf16, tag="ones")
        nc.vector.tensor_copy(ones_t[:], ones32[:])
        ident = const.tile([128, 128], f32, tag="ident")
        make_identity(nc, ident[:])
        mask_sb = const.tile([128, nmask, mw], mdt, tag="mask")
        mask_idx = {(qb, kb): i for i, (qb, kb, _) in enumerate(mask_tiles)}

        for rep in range(repeat):
            _build_body(nc, tc, mybir, dt, ctx, dram, rep, repeat, mask_desc,
                        mask_binary, mask_idx, mask_sb, cos_t, sin_t, ones_t,
                        ident, xt_in, wq_in, wk_in, wv_in, wo_in, cos_in,
                        sin_in, mask_in, out_ext, KT, NXT, XKT)

        ctx.close()

    _split_multi_waits(nc)
    return nc, [t[:2] for t in mask_tiles], mw


def _build_body(nc, tc, mybir, dt, ctx, dram, rep, repeat, mask_desc,
                mask_binary, mask_idx, mask_sb, cos_t, sin_t, ones_t, ident,
                xt_in, wq_in, wk_in, wv_in, wo_in, cos_in, sin_in, mask_in,
                out_ext, KT, NXT, XKT):
    from contextlib import ExitStack

    f32, f32r, bf16 = dt.float32, dt.float32r, dt.bfloat16

    rctx = ExitStack()
    qkvpool = rctx.enter_context(tc.tile_pool(name=f"qkv{rep}", bufs=1))

    # AG buffers: per token half, per-core heads in [feat, tok] layout.
    ag_in = [dram.tile([HL * HD, QB], bf16, tag=f"agi{rep}_{i}",
                       name=f"ag_in{rep}_{i}")
             for i in range(NHALF)]
    ag_out = [dram.tile([TPG * HL * HD, QB], bf16, tag=f"ago{rep}_{i}",
                        name=f"ag_out{rep}_{i}")
              for i in range(NHALF)]

    # activations that live through phase 2
    q_rot = [qkvpool.tile([128, S], f32r, tag=f"q{h}", name=f"q_rot{rep}_{h}")
             for h in range(HL)]
    k_rot = [qkvpool.tile([128, S], f32r, tag=f"k{j}", name=f"k_rot{rep}_{j}")
             for j in range(KVL)]
    v_nat = [qkvpool.tile([128, S // 128, 128], bf16, tag=f"v{j}",
                          name=f"v_nat{rep}_{j}")
             for j in range(KVL)]

    # ---------------- phase 1: QKV projections + rotary -----------------
    with tc.tile_pool(name=f"p1x{rep}", bufs=1) as xpool, \
         tc.tile_pool(name=f"p1w{rep}", bufs=4) as wpool, \
         tc.tile_pool(name=f"p1t{rep}", bufs=2) as tpool, \
         tc.tile_pool(name=f"p1ps{rep}", bufs=4, space="PSUM") as pspool, \
         tc.tile_pool(name=f"p1pst{rep}", bufs=2, space="PSUM") as pstr:

        # DMA issue order matters: the first matmuls need wk ct0 and the
        # leading x chunks, so interleave the k/v weight loads with
        # fine-grained x chunks instead of monolithic 2MB x DMAs.
        xt = [xpool.tile([128, XKT, S], bf16, tag=f"xt{i}",
                         name=f"xt{rep}_{i}")
              for i in range(NXT)]
        w_kv = [wpool.tile([128, KT, 128], bf16, tag="w", name=f"wkv{rep}_{j}")
                for j in range(2 * KVL)]
        nc.sync.dma_start(out=w_kv[0][:], in_=wk_in[0])
        for i in range(NXT):
            nc.sync.dma_start(
                out=xt[i][:],
                in_=xt_in[i * XKT:(i + 1) * XKT, :, :]
                    .rearrange("k p t -> p k t"),
            )
            if i == 0:
                nc.sync.dma_start(out=w_kv[1][:], in_=wk_in[1])
            elif i == 1 and rep == 0:
                nc.sync.dma_start(out=cos_t[:], in_=cos_in[:])
                nc.sync.dma_start(out=sin_t[:], in_=sin_in[:])
                nc.sync.dma_start(
                    out=mask_sb[:],
                    in_=mask_in[:].rearrange("b p c -> p b c"))
            elif i == 2:
                nc.sync.dma_start(out=w_kv[2][:], in_=wv_in[0])
            elif i == 3:
                nc.sync.dma_start(out=w_kv[3][:], in_=wv_in[1])

        def xt_sl(kt, tb):
            return xt[kt // XKT][:, kt % XKT, tb * QB:(tb + 1) * QB]

        # (dram tensor, n col tiles, kind, preloaded tiles or None)
        projs = [(wk_in, KVL, "k", w_kv[0:KVL]), (wv_in, KVL, "v", w_kv[KVL:]),
                 (wq_in, HL, "q", None)]
        for w_dram, ncts, kind, pre in projs:
            for ct in range(ncts):
                if pre is not None:
                    w_sb = pre[ct]
                else:
                    w_sb = wpool.tile([128, KT, 128], bf16, tag="w")
                    nc.sync.dma_start(out=w_sb[:], in_=w_dram[ct])
                for tb in range(S // QB):
                    ps = pspool.tile([128, QB], f32, tag="ps_qkv")
                    for kt in range(KT):
                        nc.tensor.matmul(
                            ps[:],
                            w_sb[:, kt, :],
                            xt_sl(kt, tb),
                            start=(kt == 0),
                            stop=(kt == KT - 1),
                        )
                    tsl = slice(tb * QB, (tb + 1) * QB)
                    if kind in ("q", "k"):
                        dest = q_rot[ct] if kind == "q" else k_rot[ct]
                        swap = tpool.tile([128, QB], f32, tag="swap")
                        nc.scalar.activation(
                            swap[0:64, :], ps[64:128, :],
                            mybir.ActivationFunctionType.Copy, scale=-1.0)
                        nc.scalar.activation(
                            swap[64:128, :], ps[0:64, :],
                            mybir.ActivationFunctionType.Copy)
                        t2 = tpool.tile([128, QB], f32, tag="t2")
                        nc.vector.tensor_tensor(
                            t2[:], ps[:], cos_t[:, tsl], mybir.AluOpType.mult)
                        t3 = tpool.tile([128, QB], f32, tag="t3")
                        nc.vector.tensor_tensor(
                            t3[:], swap[:], sin_t[:, tsl], mybir.AluOpType.mult)
                        nc.vector.tensor_tensor(
                            dest[:, tsl], t2[:], t3[:], mybir.AluOpType.add)
                    else:  # v: transpose to natural [t, d] layout
                        vt = tpool.tile([128, QB], f32, tag="vt")
                        nc.scalar.activation(
                            vt[:], ps[:], mybir.ActivationFunctionType.Copy)
                        for j in range(QB // 128):
                            ps_t = pstr.tile([128, 128], f32, tag="ps_tr")
                            nc.tensor.transpose(
                                ps_t[:], vt[:, j * 128:(j + 1) * 128], ident[:])
                            nc.vector.tensor_copy(
                                v_nat[ct][:, tb * (QB // 128) + j, :], ps_t[:])

    # ---------------- phase 2: attention, AG fired per token half -------
    # wo weights prefetched during attention (resident bf16
    # [p(feat within ct), ct, hid-quarter]).
    wopool = rctx.enter_context(tc.tile_pool(name=f"wo{rep}", bufs=1))
    wo_sb = wopool.tile([128, KT, HIDL], bf16, tag="wo", name=f"wo_sb{rep}")
    nc.sync.dma_start(
        out=wo_sb[:], in_=wo_in[:, :, :].rearrange("c p n -> p c n"))

    with tc.tile_pool(name=f"p2p{rep}", bufs=4) as ppool, \
         tc.tile_pool(name=f"p2r{rep}", bufs=2) as rpool, \
         tc.tile_pool(name=f"p2o{rep}", bufs=4) as aopool, \
         tc.tile_pool(name=f"p3a{rep}", bufs=3) as agpool, \
         tc.tile_pool(name=f"p3o{rep}", bufs=4) as opool:

        def ag_chunk_dma(half, tl):
            t = agpool.tile([128, KT, 128], bf16, tag="ag",
                            name=f"ag_sb{rep}_{half}_{tl}")
            # issued on the ACT HWDGE ring: the sync ring is full of
            # attention-output DMAs that become ready later, and FIFO
            # head-of-line blocking there would stall the wo start.
            nc.scalar.dma_start(
                out=t[:],
                in_=ag_out[half][:, tl * 128:(tl + 1) * 128]
                    .rearrange("(c p) t -> p c t", p=128))
            return t

        prefetched = {}

        attnps = ExitStack()
        scpool = attnps.enter_context(
            tc.tile_pool(name=f"p2sc{rep}", bufs=2, space="PSUM"))
        pvpool = attnps.enter_context(
            tc.tile_pool(name=f"p2pv{rep}", bufs=2, space="PSUM"))
        dnpool = attnps.enter_context(
            tc.tile_pool(name=f"p2dn{rep}", bufs=2, space="PSUM"))
        pso = attnps.enter_context(
            tc.tile_pool(name=f"p3ps{rep}", bufs=2, space="PSUM"))

        # token half 1 first: its attention is PE-dense and its AG then
        # overlaps half 0's DVE/ACT-heavy attention, so the wo phase can
        # start the moment attention ends.
        for qb in reversed(range(NHALF)):
            for h in range(HL):
                kvh = h // GQ
                blocks = []  # (kb, off, need_mask)
                for kb in range(S // 128):
                    d = mask_desc[qb][kb]
                    if d[0] == "skip":
                        continue
                    if d[0] == "causal":
                        blocks.append((kb, d[1], ("diag", d[1])))
                    else:
                        blocks.append((kb, 0, ("full",) if d[1] else None))
                ps_pv = pvpool.tile([128, QB], f32, tag="ps_pv")
                ps_dn = dnpool.tile([128, QB], f32, tag="ps_dn")
                nblk = len(blocks)
                for bi, (kb, off, mk) in enumerate(blocks):
                    qsl = slice(qb * QB + off, (qb + 1) * QB)
                    ps_sc = scpool.tile([128, QB], f32, tag="ps_sc")
                    nc.tensor.matmul(
                        ps_sc[:, off:QB],
                        k_rot[kvh][:, kb * 128:(kb + 1) * 128],
                        q_rot[h][:, qsl],
                        start=True, stop=True,
                    )
                    if mk is not None and not mask_binary:
                        mi = mask_idx[(qb, kb)]
                        if mk[0] == "diag":
                            nc.vector.tensor_tensor(
                                ps_sc[:, off:off + 128], ps_sc[:, off:off + 128],
                                mask_sb[:, mi, 0:128], mybir.AluOpType.add)
                        else:
                            nc.vector.tensor_tensor(
                                ps_sc[:, 0:QB], ps_sc[:, 0:QB],
                                mask_sb[:, mi, 0:QB], mybir.AluOpType.add)
                    p_t = ppool.tile([128, QB], bf16, tag="p")
                    nc.scalar.activation(
                        p_t[:, off:QB], ps_sc[:, off:QB],
                        mybir.ActivationFunctionType.Exp)
                    if mk is not None and mask_binary:
                        # exp(s+m) == exp(s) * [m == 0]: zero the masked
                        # entries after exp, off the scores->exp chain
                        mi = mask_idx[(qb, kb)]
                        if mk[0] == "diag":
                            nc.vector.tensor_tensor(
                                p_t[:, off:off + 128], p_t[:, off:off + 128],
                                mask_sb[:, mi, 0:128], mybir.AluOpType.mult)
                        else:
                            nc.vector.tensor_tensor(
                                p_t[:, 0:QB], p_t[:, 0:QB],
                                mask_sb[:, mi, 0:QB], mybir.AluOpType.mult)
                    nc.tensor.matmul(
                        ps_pv[:, off:QB],
                        v_nat[kvh][:, kb, :],
                        p_t[:, off:QB],
                        start=(bi == 0), stop=(bi == nblk - 1),
                    )
                    nc.tensor.matmul(
                        ps_dn[:, off:QB],
                        ones_t[:],
                        p_t[:, off:QB],
                        start=(bi == 0), stop=(bi == nblk - 1),
                    )
                recip = rpool.tile([128, QB], f32, tag="recip")
                nc.vector.reciprocal(recip[:], ps_dn[:])
                o_attn = aopool.tile([128, QB], bf16, tag="oat")
                nc.vector.tensor_tensor(
                    o_attn[:], ps_pv[:], recip[:], mybir.AluOpType.mult)
                nc.sync.dma_start(
                    out=ag_in[qb][h * HD:(h + 1) * HD, :], in_=o_attn[:])

            # all heads of this token half written: gather within group
            nc.gpsimd.collective_compute(
                "AllGather", mybir.AluOpType.bypass,
                replica_groups=[list(range(g * TPG, (g + 1) * TPG))
                                for g in range(NGROUPS)],
                ins=[ag_in[qb].opt()], outs=[ag_out[qb].opt()],
            )
            if qb == NHALF - 1:
                # prefetch the first wo chunks right behind this AG, ahead
                # of half 0's output DMAs in the queue (avoids
                # head-of-line blocking on the wo start).
                for tl in range(2):
                    prefetched[(qb, tl)] = ag_chunk_dma(qb, tl)

        # ------------- phase 3: output projection -----------------------
        NHB = HIDL // QB  # 2 hid blocks of 512
        for half in reversed(range(NHALF)):
            for tl in range(QB // 128):           # token tiles within half
                tt = half * (QB // 128) + tl
                ag_sb = prefetched.pop((half, tl), None)
                if ag_sb is None:
                    ag_sb = ag_chunk_dma(half, tl)
                for hb in range(NHB):
                    ps_o = pso.tile([128, QB], f32, tag="ps_o")
                    for ct in range(KT):
                        nc.tensor.matmul(
                            ps_o[:],
                            ag_sb[:, ct, :],
                            wo_sb[:, ct, hb * QB:(hb + 1) * QB],
                            start=(ct == 0), stop=(ct == KT - 1),
                        )
                    o_sb = opool.tile([128, QB], f32, tag="o")
                    if (hb + tt) % 2 == 0:
                        nc.vector.tensor_copy(o_sb[:], ps_o[:])
                    else:
                        nc.scalar.activation(
                            o_sb[:], ps_o[:],
                            mybir.ActivationFunctionType.Copy)
                    nc.sync.dma_start(
                        out=out_ext[tt * 128:(tt + 1) * 128,
                                    hb * QB:(hb + 1) * QB],
                        in_=o_sb[:])
        attnps.close()

    rctx.close()


# ----------------------------------------------------------------------------
# Host-side input prep
# ----------------------------------------------------------------------------
def _classify_mask(attn_mask):
    """Per (qb, kb) descriptor from the actual mask contents (transposed
    [kv, q] view). Causal masks produce the efficient windowed structure."""
    mt = attn_mask.T  # [kv, q]
    desc = []
    for qb in range(S // QB):
        row = []
        q0 = qb * QB
        for kb in range(S // 128):
            blk = mt[kb * 128:(kb + 1) * 128, q0:q0 + QB]
            if np.all(blk <= NEG_THRESH):
                row.append(("skip",))
                continue
            if np.all(np.abs(blk) < 1e-6):
                row.append(("full", False))
                continue
            # causal window? cols [0, off) fully masked, diag at [off, off+128),
            # cols beyond fully visible
            off = kb * 128 - q0
            causal = False
            if 0 <= off <= QB - 128:
                left_ok = np.all(blk[:, :off] <= NEG_THRESH) if off else True
                right_ok = (np.all(np.abs(blk[:, off + 128:]) < 1e-6)
                            if off + 128 < QB else True)
                causal = bool(left_ok and right_ok)
            if causal:
                row.append(("causal", off))
            else:
                row.append(("full", True))
        desc.append(row)
    # every q column must keep at least one contributing block
    for qb in range(S // QB):
        assert any(d[0] != "skip" for d in desc[qb]), "fully-masked q rows unsupported"
    return desc


def _mask_is_binary(attn_mask):
    """True when every mask entry is (near) 0 or fully-masked, so masking can
    run as a post-exp multiply by 0/1."""
    return bool(np.all((np.abs(attn_mask) < 1e-6) | (attn_mask <= NEG_THRESH)))


def _prep_core_inputs(inputs, mask_desc, mask_list, mw, mask_binary):
    x = np.asarray(inputs["x"], np.float32)
    wq = np.asarray(inputs["wq"], np.float32)
    wk = np.asarray(inputs["wk"], np.float32)
    wv = np.asarray(inputs["wv"], np.float32)
    wo = np.asarray(inputs["wo"], np.float32)
    attn_mask = np.asarray(inputs["attn_mask"], np.float32)
    start_pos = np.asarray(inputs["start_pos"], np.int32)

    bf = ml_dtypes.bfloat16
    KT = HID // 128

    inv_freq = 1.0 / (10000.0 ** (np.arange(0, HD, 2, dtype=np.float32) / HD))
    mt = attn_mask.T
    if mask_list:
        mask_arr = np.zeros((len(mask_list), 128, mw), np.float32)
        for i, (qb, kb) in enumerate(mask_list):
            d = mask_desc[qb][kb]
            if d[0] == "causal":
                off = d[1]
                mask_arr[i, :, 0:128] = mt[kb * 128:(kb + 1) * 128,
                                           qb * QB + off:qb * QB + off + 128]
            else:
                mask_arr[i, :, 0:QB] = mt[kb * 128:(kb + 1) * 128,
                                          qb * QB:(qb + 1) * QB]
    else:
        mask_arr = np.zeros((1, 128, mw), np.float32)
    if mask_binary:
        mask_arr = (mask_arr > NEG_THRESH).astype(ml_dtypes.bfloat16)

    # lhsT tile layout: [ct, p=hid_within_kt, kt, col_within_ct]
    def wtile2(w):
        c = w.shape[1]
        return np.ascontiguousarray(
            w.reshape(KT, 128, c // 128, 128).transpose(2, 1, 0, 3))

    in_maps = []
    for core in range(NCORES):
        g, r = divmod(core, TPG)
        xb = x[g * S:(g + 1) * S]                       # [S, HID]
        xt = np.ascontiguousarray(xb.T.reshape(KT, 128, S)).astype(bf)
        wq_c = (wq[:, r * HL * HD:(r + 1) * HL * HD] * SCALE)
        wk_c = wk[:, r * KVL * HD:(r + 1) * KVL * HD]
        wv_c = wv[:, r * KVL * HD:(r + 1) * KVL * HD]
        wo_c = wo[:, r * HIDL:(r + 1) * HIDL]           # [4096, 1024]

        pos = start_pos[g] + np.arange(S, dtype=np.float32)
        ang = pos[:, None] * inv_freq[None, :]          # [S, HD/2]
        cos = np.concatenate([np.cos(ang), np.cos(ang)], -1).T  # [HD, S]
        sin = np.concatenate([np.sin(ang), np.sin(ang)], -1).T

        in_maps.append({
            "xt": xt,
            "wq": wtile2(wq_c).astype(bf),
            "wk": wtile2(wk_c).astype(bf),
            "wv": wtile2(wv_c).astype(bf),
            "wo": np.ascontiguousarray(wo_c.reshape(KT, 128, HIDL)).astype(bf),
            "cos_t": np.ascontiguousarray(cos.astype(np.float32)),
            "sin_t": np.ascontiguousarray(sin.astype(np.float32)),
            "maskt": mask_arr,
        })
    return in_maps


def _make_runner(nc):
    """Cached jit over the bass module (adapted from
    concourse.bass2jax.run_bass_via_pjrt so repeat calls reuse one NEFF)."""
    import jax
    import jax.numpy as jnp
    from jax.sharding import Mesh, NamedSharding, PartitionSpec
    from jax.experimental.shard_map import shard_map

    import concourse.mybir as mybir
    from concourse import bass2jax

    bass2jax.install_neuronx_cc_hook()
    assert nc.dbg_addr is None
    partition_name = (nc.partition_id_tensor.name
                      if nc.partition_id_tensor else None)

    in_names, out_names, out_avals, out_shapes = [], [], [], []
    for alloc in nc.m.functions[0].allocations:
        if not isinstance(alloc, mybir.MemoryLocationSet):
            continue
        name = alloc.memorylocations[0].name
        if alloc.kind == "ExternalInput":
            if name != partition_name:
                in_names.append(name)
        elif alloc.kind == "ExternalOutput":
            assert alloc.tensor_shape is not None and alloc.dtype is not None
            shape = tuple(alloc.tensor_shape)
            npdt = mybir.dt.np(alloc.dtype)
            out_names.append(name)
            out_shapes.append((shape, npdt))
            out_avals.append(jax.core.ShapedArray(shape, npdt))

    n_params = len(in_names)
    n_outs = len(out_names)
    all_in_names = in_names + out_names
    if partition_name is not None:
        all_in_names = all_in_names + [partition_name]
    donate = tuple(range(n_params, n_params + n_outs))

    def _body(*args):
        operands = list(args)
        if partition_name is not None:
            operands.append(bass2jax.partition_id_tensor())
        outs = bass2jax._bass_exec_p.bind(
            *operands,
            out_avals=tuple(out_avals),
            in_names=tuple(all_in_names),
            out_names=tuple(out_names),
            lowering_input_output_aliases=(),
            sim_require_finite=True,
            sim_require_nnan=True,
            nc=nc,
        )
        return tuple(outs)

    devices = jax.devices()[:NCORES]
    mesh = Mesh(np.asarray(devices), ("core",))
    pc = PartitionSpec("core")
    sharded = jax.jit(
        shard_map(_body, mesh=mesh, in_specs=(pc,) * (n_params + n_outs),
                  out_specs=(pc,) * n_outs, check_rep=False),
        donate_argnums=donate, keep_unused=True)

    shard_dev = NamedSharding(mesh, pc)

    def make_zeros():
        return tuple(
            jax.device_put(np.zeros((NCORES * s[0], *s[1:]), d), shard_dev)
            for s, d in out_shapes)

    def put_inputs(in_maps):
        return [
            jax.device_put(
                np.concatenate([np.asarray(m[nm]) for m in in_maps], axis=0),
                shard_dev)
            for nm in in_names]

    def run_from_dev(in_dev, zeros):
        out_arrs = sharded(*in_dev, *zeros)
        jax.block_until_ready(out_arrs)
        return out_arrs

    def run(in_maps):
        out_arrs = run_from_dev(put_inputs(in_maps), make_zeros())
        return [
            {nm: np.asarray(out_arrs[i]).reshape(NCORES, *out_shapes[i][0])[c]
             for i, nm in enumerate(out_names)}
            for c in range(NCORES)]

    return {"run": run, "put_inputs": put_inputs, "make_zeros": make_zeros,
            "run_from_dev": run_from_dev, "sharded": sharded}


def _get_runner(mask_desc, mask_binary, repeat=1):
    key = (repr(mask_desc), mask_binary, repeat)
    cache = _STATE.setdefault("runners", {})
    if key not in cache:
        nc, mask_list, mw = _build_module(mask_desc, mask_binary, repeat)
        runner = _make_runner(nc)
        cache[key] = {"runner": runner, "mask_list": mask_list, "mw": mw,
                      "nc": nc}
    e = cache[key]
    _STATE.update({"run": e["runner"]["run"], "mask_list": e["mask_list"],
                   "mw": e["mw"], "nc": e["nc"], "runner": e["runner"]})
    return e["runner"]["run"], e["mask_list"], e["mw"]


def kernel(**inputs) -> np.ndarray:
    attn_mask = np.asarray(inputs["attn_mask"], np.float32)
    mask_desc = _classify_mask(attn_mask)
    mask_binary = _mask_is_binary(attn_mask)
    run, mask_list, mw = _get_runner(mask_desc, mask_binary)
    in_maps = _prep_core_inputs(inputs, mask_desc, mask_list, mw, mask_binary)
    results = run(in_maps)
    out = np.empty((B * S, HID), np.float32)
    for core in range(NCORES):
        g, r = divmod(core, TPG)
        out[g * S:(g + 1) * S, r * HIDL:(r + 1) * HIDL] = results[core]["outp"]
    return out


# revision 39
# speedup vs baseline: 2.3519x; 1.4812x over previous
"""Trainium2 Bass kernel for GQA attention (B=2, S=1024, HID=4096, H=32,
HKV=8, HD=128) with NeoX rotary + additive mask, sharded over 8 NeuronCores.

Sharding: 2 data-parallel groups (one per batch sequence) x 4-way tensor
parallel (8 q-heads / 2 kv-heads per core). wq/wk/wv column-sharded.
After attention, the per-core head outputs (bf16, [1024 feat, tok]) are
AllGathered within each 4-core group (2 x 1MB AG per group, fired per
512-token half so the first overlaps the second half's attention), then each
core computes the output projection over the FULL 4096-feature contraction
for its quarter of the HID columns (wo column-sharded). No ReduceScatter and
no fp32 partial-sum traffic; the host concatenates the 8 disjoint
[1024 x 1024] output blocks.

Everything on device runs in a transposed layout ([feature, token]) so every
matmul streams with free-dim 512 at full PE rate (fp32r for attention scores,
bf16 inputs for QKV/PV/wo with fp32 PSUM accumulation).

`repeat` builds a module that runs the whole forward pass N times back to
back; timing R>1 against R=1 isolates true device execution time from the
per-dispatch host overhead (~350us on this axon-tunneled setup).
"""

import math

import ml_dtypes
import numpy as np

B, S, HID, H, HKV, HD = 2, 1024, 4096, 32, 8, 128
NCORES = 8
TPG = 4                      # tensor-parallel group size
NGROUPS = NCORES // TPG      # data-parallel groups (= B)
HL = H // TPG                # q heads per core (8)
KVL = HKV // TPG             # kv heads per core (2)
GQ = H // HKV                # q heads per kv head (4)
SCALE = 1.0 / math.sqrt(HD)
QB = 512                     # q block (free dim of attention matmuls)
NHALF = S // QB              # token halves (2)
HIDL = HID // TPG            # output columns per core (1024)
NEG_THRESH = -1.0e8          # mask values <= this count as fully masked

_STATE: dict = {}


# ----------------------------------------------------------------------------
# walrus compat: this toolchain supports at most ONE semaphore wait per
# instruction; Tile's scheduler can attach several. Hoist extras onto
# same-engine nops placed immediately before the instruction.
# ----------------------------------------------------------------------------
def _split_multi_waits(nc):
    import concourse.mybir as mybir

    def detached_nop(engine_type):
        bi = nc.engines[engine_type].nop()
        inst = bi.ins
        for fn in nc.m.functions:
            for b in fn.blocks:
                il = b.instructions
                if il and il[-1].name == inst.name:
                    il.pop()
                    return inst
        raise AssertionError("could not detach nop")

    for fn in nc.m.functions:
        for b in fn.blocks:
            il = b.instructions
            out = []
            changed = False
            for inst in il:
                si = inst.sync_info
                waits = list(si.on_wait) if (si is not None and si.on_wait) else []
                if len(waits) > 1:
                    for w in waits[:-1]:
                        nop = detached_nop(inst.engine)
                        nop.sync_info = mybir.SyncInfo(on_wait=[w], on_update=[])
                        out.append(nop)
                    si.on_wait = waits[-1:]
                    changed = True
                out.append(inst)
            if changed:
                b.instructions = out


# ----------------------------------------------------------------------------
# Device program
# ----------------------------------------------------------------------------
def _build_module(mask_desc, mask_binary, repeat=1):
    """mask_desc: per (qb, kb) block descriptor list computed on the host from
    the actual attn_mask:
      ("skip",)                 block fully masked
      ("full", need_mask:bool)  full 512-wide block, optionally + mask data
      ("causal", off:int)       causal window: cols [off,512) active, mask
                                on the 128-wide diagonal window at `off`
    mask_binary: True when every mask value is 0 or -inf-like; masking is then
    a post-exp multiply by a 0/1 bf16 mask (off the scores critical path).
    Otherwise the mask is added to the scores in PSUM before exp (exact for
    arbitrary additive masks).
    """
    import concourse.bass as bass
    import concourse.mybir as mybir
    import concourse.tile as tile
    from concourse.masks import make_identity
    from contextlib import ExitStack

    dt = mybir.dt
    f32, f32r, bf16 = dt.float32, dt.float32r, dt.bfloat16
    KT = HID // 128  # 32 contraction tiles
    NXT = 8          # x chunks (4 kt tiles each)
    XKT = KT // NXT

    nc = bass.Bass()

    # --- DRAM parameters (per-core shards, host-prepared) ---
    xt_in = nc.declare_dram_parameter("xt", [KT, 128, S], bf16, isOutput=False)
    wq_in = nc.declare_dram_parameter("wq", [HL, 128, KT, 128], bf16, isOutput=False)
    wk_in = nc.declare_dram_parameter("wk", [KVL, 128, KT, 128], bf16, isOutput=False)
    wv_in = nc.declare_dram_parameter("wv", [KVL, 128, KT, 128], bf16, isOutput=False)
    wo_in = nc.declare_dram_parameter("wo", [KT, 128, HIDL], bf16, isOutput=False)
    cos_in = nc.declare_dram_parameter("cos_t", [128, S], f32, isOutput=False)
    sin_in = nc.declare_dram_parameter("sin_t", [128, S], f32, isOutput=False)
    # mask blocks actually referenced by the program, in transposed [kv, q]
    # layout; index map built below.
    mask_tiles = []
    for qb in range(NHALF):
        for kb in range(S // 128):
            d = mask_desc[qb][kb]
            if d[0] == "causal":
                mask_tiles.append((qb, kb, 128))
            elif d[0] == "full" and d[1]:
                mask_tiles.append((qb, kb, QB))
    nmask = max(1, len(mask_tiles))
    mw = max([t[2] for t in mask_tiles], default=128)
    mdt = bf16 if mask_binary else f32
    mask_in = nc.declare_dram_parameter("maskt", [nmask, 128, mw], mdt, isOutput=False)
    out_ext = nc.declare_dram_parameter("outp", [S, HIDL], f32, isOutput=True)

    ctx = ExitStack()
    with tile.TileContext(nc) as tc:
        const = ctx.enter_context(tc.tile_pool(name="const", bufs=1))
        dram = ctx.enter_context(tc.tile_pool(name="dram", bufs=1, space="DRAM"))

        cos_t = const.tile([128, S], f32, tag="cos")
        sin_t = const.tile([128, S], f32, tag="sin")
        ones32 = const.tile([128, 128], f32, tag="ones32")
        nc.gpsimd.memset(ones32[:], 1.0)
        ones_t = const.tile([128, 128], bf16, tag="ones")
        nc.vector.tensor_copy(ones_t[:], ones32[:])
        ident = const.tile([128, 128], f32, tag="ident")
        make_identity(nc, ident[:])
        mask_sb = const.tile([128, nmask, mw], mdt, tag="mask")
        mask_idx = {(qb, kb): i for i, (qb, kb, _) in enumerate(mask_tiles)}

        for rep in range(repeat):
            _build_body(nc, tc, mybir, dt, ctx, dram, rep, repeat, mask_desc,
                        mask_binary, mask_idx, mask_sb, cos_t, sin_t, ones_t,
                        ident, xt_in, wq_in, wk_in, wv_in, wo_in, cos_in,
                        sin_in, mask_in, out_ext, KT, NXT, XKT)

        ctx.close()

    _split_multi_waits(nc)
    return nc, [t[:2] for t in mask_tiles], mw


def _build_body(nc, tc, mybir, dt, ctx, dram, rep, repeat, mask_desc,
                mask_binary, mask_idx, mask_sb, cos_t, sin_t, ones_t, ident,
                xt_in, wq_in, wk_in, wv_in, wo_in, cos_in, sin_in, mask_in,
                out_ext, KT, NXT, XKT):
    from contextlib import ExitStack

    f32, f32r, bf16 = dt.float32, dt.float32r, dt.bfloat16

    rctx = ExitStack()
    qkvpool = rctx.enter_context(tc.tile_pool(name=f"qkv{rep}", bufs=1))

    # AG buffers: per token half, per-core heads in [feat, tok] layout.
    ag_in = [dram.tile([HL * HD, QB], bf16, tag=f"agi{rep}_{i}",
                       name=f"ag_in{rep}_{i}")
             for i in range(NHALF)]
    ag_out = [dram.tile([TPG * HL * HD, QB], bf16, tag=f"ago{rep}_{i}",
                        name=f"ag_out{rep}_{i}")
              for i in range(NHALF)]

    # activations that live through phase 2
    q_rot = [qkvpool.tile([128, S], f32r, tag=f"q{h}", name=f"q_rot{rep}_{h}")
             for h in range(HL)]
    k_rot = [qkvpool.tile([128, S], f32r, tag=f"k{j}", name=f"k_rot{rep}_{j}")
             for j in range(KVL)]
    v_nat = [qkvpool.tile([128, S // 128, 128], bf16, tag=f"v{j}",
                          name=f"v_nat{rep}_{j}")
             for j in range(KVL)]

    # ---------------- phase 1: QKV projections + rotary -----------------
    with tc.tile_pool(name=f"p1x{rep}", bufs=1) as xpool, \
         tc.tile_pool(name=f"p1w{rep}", bufs=4) as wpool, \
         tc.tile_pool(name=f"p1t{rep}", bufs=2) as tpool, \
         tc.tile_pool(name=f"p1ps{rep}", bufs=4, space="PSUM") as pspool, \
         tc.tile_pool(name=f"p1pst{rep}", bufs=2, space="PSUM") as pstr:

        # DMA issue order matters: the first matmuls need wk ct0 and the
        # leading x chunks, so interleave the k/v weight loads with
        # fine-grained x chunks instead of monolithic 2MB x DMAs.
        xt = [xpool.tile([128, XKT, S], bf16, tag=f"xt{i}",
                         name=f"xt{rep}_{i}")
              for i in range(NXT)]
        w_kv = [wpool.tile([128, KT, 128], bf16, tag="w", name=f"wkv{rep}_{j}")
                for j in range(2 * KVL)]
        nc.sync.dma_start(out=w_kv[0][:], in_=wk_in[0])
        for i in range(NXT):
            nc.sync.dma_start(
                out=xt[i][:],
                in_=xt_in[i * XKT:(i + 1) * XKT, :, :]
                    .rearrange("k p t -> p k t"),
            )
            if i == 0:
                nc.sync.dma_start(out=w_kv[1][:], in_=wk_in[1])
            elif i == 1 and rep == 0:
                nc.sync.dma_start(out=cos_t[:], in_=cos_in[:])
                nc.sync.dma_start(out=sin_t[:], in_=sin_in[:])
                nc.sync.dma_start(
                    out=mask_sb[:],
                    in_=mask_in[:].rearrange("b p c -> p b c"))
            elif i == 2:
                nc.sync.dma_start(out=w_kv[2][:], in_=wv_in[0])
            elif i == 3:
                nc.sync.dma_start(out=w_kv[3][:], in_=wv_in[1])

        def xt_sl(kt, tb):
            return xt[kt // XKT][:, kt % XKT, tb * QB:(tb + 1) * QB]

        # (dram tensor, n col tiles, kind, preloaded tiles or None)
        projs = [(wk_in, KVL, "k", w_kv[0:KVL]), (wv_in, KVL, "v", w_kv[KVL:]),
                 (wq_in, HL, "q", None)]
        for w_dram, ncts, kind, pre in projs:
            for ct in range(ncts):
                if pre is not None:
                    w_sb = pre[ct]
                else:
                    w_sb = wpool.tile([128, KT, 128], bf16, tag="w")
                    nc.sync.dma_start(out=w_sb[:], in_=w_dram[ct])
                for tb in range(S // QB):
                    ps = pspool.tile([128, QB], f32, tag="ps_qkv")
                    for kt in range(KT):
                        nc.tensor.matmul(
                            ps[:],
                            w_sb[:, kt, :],
                            xt_sl(kt, tb),
                            start=(kt == 0),
                            stop=(kt == KT - 1),
                        )
                    tsl = slice(tb * QB, (tb + 1) * QB)
                    if kind in ("q", "k"):
                        dest = q_rot[ct] if kind == "q" else k_rot[ct]
                        swap = tpool.tile([128, QB], f32, tag="swap")
                        nc.scalar.activation(
                            swap[0:64, :], ps[64:128, :],
                            mybir.ActivationFunctionType.Copy, scale=-1.0)
                        nc.scalar.activation(
                            swap[64:128, :], ps[0:64, :],
                            mybir.ActivationFunctionType.Copy)
                        t2 = tpool.tile([128, QB], f32, tag="t2")
                        nc.vector.tensor_tensor(
                            t2[:], ps[:], cos_t[:, tsl], mybir.AluOpType.mult)
                        t3 = tpool.tile([128, QB], f32, tag="t3")
                        nc.vector.tensor_tensor(
                            t3[:], swap[:], sin_t[:, tsl], mybir.AluOpType.mult)
                        nc.vector.tensor_tensor(
                            dest[:, tsl], t2[:], t3[:], mybir.AluOpType.add)
                    else:  # v: transpose to natural [t, d] layout
                        vt = tpool.tile([128, QB], f32, tag="vt")
                        nc.scalar.activation(
                            vt[:], ps[:], mybir.ActivationFunctionType.Copy)
                        for j in range(QB // 128):
                            ps_t = pstr.tile([128, 128], f32, tag="ps_tr")
                            nc.tensor.transpose(
                                ps_t[:], vt[:, j * 128:(j + 1) * 128], ident[:])
                            nc.vector.tensor_copy(
                                v_nat[ct][:, tb * (QB // 128) + j, :], ps_t[:])

    # ---------------- phase 2: attention, AG fired per token half -------
    # wo weights prefetched during attention (resident bf16
    # [p(feat within ct), ct, hid-quarter]).
    wopool = rctx.enter_context(tc.tile_pool(name=f"wo{rep}", bufs=1))
    wo_sb = wopool.tile([128, KT, HIDL], bf16, tag="wo", name=f"wo_sb{rep}")
    nc.sync.dma_start(
        out=wo_sb[:], in_=wo_in[:, :, :].rearrange("c p n -> p c n"))

    with tc.tile_pool(name=f"p2p{rep}", bufs=4) as ppool, \
         tc.tile_pool(name=f"p2r{rep}", bufs=2) as rpool, \
         tc.tile_pool(name=f"p2o{rep}", bufs=4) as aopool, \
         tc.tile_pool(name=f"p3a{rep}", bufs=3) as agpool, \
         tc.tile_pool(name=f"p3o{rep}", bufs=4) as opool:

        def ag_chunk_dma(half, tl, eng=None):
            t = agpool.tile([128, KT, 128], bf16, tag="ag",
                            name=f"ag_sb{rep}_{half}_{tl}")
            # default ACT HWDGE ring; the first prefetched chunk goes on the
            # sync ring instead, since ACT stays saturated with attention
            # exps until the very end and would dispatch it too late.
            (eng or nc.scalar).dma_start(
                out=t[:],
                in_=ag_out[half][:, tl * 128:(tl + 1) * 128]
                    .rearrange("(c p) t -> p c t", p=128))
            return t

        prefetched = {}

        attnps = ExitStack()
        scpool = attnps.enter_context(
            tc.tile_pool(name=f"p2sc{rep}", bufs=2, space="PSUM"))
        pvpool = attnps.enter_context(
            tc.tile_pool(name=f"p2pv{rep}", bufs=2, space="PSUM"))
        dnpool = attnps.enter_context(
            tc.tile_pool(name=f"p2dn{rep}", bufs=2, space="PSUM"))
        pso = attnps.enter_context(
            tc.tile_pool(name=f"p3ps{rep}", bufs=2, space="PSUM"))

        # token half 1 first: its attention is PE-dense and its AG then
        # overlaps half 0's DVE/ACT-heavy attention, so the wo phase can
        # start the moment attention ends.
        for qb in reversed(range(NHALF)):
            for h in range(HL):
                kvh = h // GQ
                blocks = []  # (kb, off, need_mask)
                for kb in range(S // 128):
                    d = mask_desc[qb][kb]
                    if d[0] == "skip":
                        continue
                    if d[0] == "causal":
                        blocks.append((kb, d[1], ("diag", d[1])))
                    else:
                        blocks.append((kb, 0, ("full",) if d[1] else None))
                ps_pv = pvpool.tile([128, QB], f32, tag="ps_pv")
                ps_dn = dnpool.tile([128, QB], f32, tag="ps_dn")
                nblk = len(blocks)
                for bi, (kb, off, mk) in enumerate(blocks):
                    qsl = slice(qb * QB + off, (qb + 1) * QB)
                    ps_sc = scpool.tile([128, QB], f32, tag="ps_sc")
                    nc.tensor.matmul(
                        ps_sc[:, off:QB],
                        k_rot[kvh][:, kb * 128:(kb + 1) * 128],
                        q_rot[h][:, qsl],
                        start=True, stop=True,
                    )
                    if mk is not None and not mask_binary:
                        mi = mask_idx[(qb, kb)]
                        if mk[0] == "diag":
                            nc.vector.tensor_tensor(
                                ps_sc[:, off:off + 128], ps_sc[:, off:off + 128],
                                mask_sb[:, mi, 0:128], mybir.AluOpType.add)
                        else:
                            nc.vector.tensor_tensor(
                                ps_sc[:, 0:QB], ps_sc[:, 0:QB],
                                mask_sb[:, mi, 0:QB], mybir.AluOpType.add)
                    p_t = ppool.tile([128, QB], bf16, tag="p")
                    nc.scalar.activation(
                        p_t[:, off:QB], ps_sc[:, off:QB],
                        mybir.ActivationFunctionType.Exp)
                    if mk is not None and mask_binary:
                        # exp(s+m) == exp(s) * [m == 0]: zero the masked
                        # entries after exp, off the scores->exp chain
                        mi = mask_idx[(qb, kb)]
                        if mk[0] == "diag":
                            nc.vector.tensor_tensor(
                                p_t[:, off:off + 128], p_t[:, off:off + 128],
                                mask_sb[:, mi, 0:128], mybir.AluOpType.mult)
                        else:
                            nc.vector.tensor_tensor(
                                p_t[:, 0:QB], p_t[:, 0:QB],
                                mask_sb[:, mi, 0:QB], mybir.AluOpType.mult)
                    nc.tensor.matmul(
                        ps_pv[:, off:QB],
                        v_nat[kvh][:, kb, :],
                        p_t[:, off:QB],
                        start=(bi == 0), stop=(bi == nblk - 1),
                    )
                    nc.tensor.matmul(
                        ps_dn[:, off:QB],
                        ones_t[:],
                        p_t[:, off:QB],
                        start=(bi == 0), stop=(bi == nblk - 1),
                    )
                recip = rpool.tile([128, QB], f32, tag="recip")
                nc.vector.reciprocal(recip[:], ps_dn[:])
                o_attn = aopool.tile([128, QB], bf16, tag="oat")
                nc.vector.tensor_tensor(
                    o_attn[:], ps_pv[:], recip[:], mybir.AluOpType.mult)
                nc.sync.dma_start(
                    out=ag_in[qb][h * HD:(h + 1) * HD, :], in_=o_attn[:])

            # all heads of this token half written: gather within group
            nc.gpsimd.collective_compute(
                "AllGather", mybir.AluOpType.bypass,
                replica_groups=[list(range(g * TPG, (g + 1) * TPG))
                                for g in range(NGROUPS)],
                ins=[ag_in[qb].opt()], outs=[ag_out[qb].opt()],
            )
            if qb == NHALF - 1:
                # prefetch the first wo chunks right behind this AG, ahead
                # of half 0's output DMAs in the queue (avoids
                # head-of-line blocking on the wo start).
                for tl in range(2):
                    prefetched[(qb, tl)] = ag_chunk_dma(qb, tl)

        # ------------- phase 3: output projection -----------------------
        NHB = HIDL // QB  # 2 hid blocks of 512
        for half in reversed(range(NHALF)):
            for tl in range(QB // 128):           # token tiles within half
                tt = half * (QB // 128) + tl
                ag_sb = prefetched.pop((half, tl), None)
                if ag_sb is None:
                    ag_sb = ag_chunk_dma(half, tl)
                for hb in range(NHB):
                    ps_o = pso.tile([128, QB], f32, tag="ps_o")
                    for ct in range(KT):
                        nc.tensor.matmul(
                            ps_o[:],
                            ag_sb[:, ct, :],
                            wo_sb[:, ct, hb * QB:(hb + 1) * QB],
                            start=(ct == 0), stop=(ct == KT - 1),
                        )
                    o_sb = opool.tile([128, QB], f32, tag="o")
                    if (hb + tt) % 2 == 0:
                        nc.vector.tensor_copy(o_sb[:], ps_o[:])
                    else:
                        nc.scalar.activation(
                            o_sb[:], ps_o[:],
                            mybir.ActivationFunctionType.Copy)
                    nc.sync.dma_start(
                        out=out_ext[tt * 128:(tt + 1) * 128,
                                    hb * QB:(hb + 1) * QB],
                        in_=o_sb[:])
        attnps.close()

    rctx.close()


# ----------------------------------------------------------------------------
# Host-side input prep
# ----------------------------------------------------------------------------
def _classify_mask(attn_mask):
    """Per (qb, kb) descriptor from the actual mask contents (transposed
    [kv, q] view). Causal masks produce the efficient windowed structure."""
    mt = attn_mask.T  # [kv, q]
    desc = []
    for qb in range(S // QB):
        row = []
        q0 = qb * QB
        for kb in range(S // 128):
            blk = mt[kb * 128:(kb + 1) * 128, q0:q0 + QB]
            if np.all(blk <= NEG_THRESH):
                row.append(("skip",))
                continue
            if np.all(np.abs(blk) < 1e-6):
                row.append(("full", False))
                continue
            # causal window? cols [0, off) fully masked, diag at [off, off+128),
            # cols beyond fully visible
            off = kb * 128 - q0
            causal = False
            if 0 <= off <= QB - 128:
                left_ok = np.all(blk[:, :off] <= NEG_THRESH) if off else True
                right_ok = (np.all(np.abs(blk[:, off + 128:]) < 1e-6)
                            if off + 128 < QB else True)
                causal = bool(left_ok and right_ok)
            if causal:
                row.append(("causal", off))
            else:
                row.append(("full", True))
        desc.append(row)
    # every q column must keep at least one contributing block
    for qb in range(S // QB):
        assert any(d[0] != "skip" for d in desc[qb]), "fully-masked q rows unsupported"
    return desc


def _mask_is_binary(attn_mask):
    """True when every mask entry is (near) 0 or fully-masked, so masking can
    run as a post-exp multiply by 0/1."""
    return bool(np.all((np.abs(attn_mask) < 1e-6) | (attn_mask <= NEG_THRESH)))


def _prep_core_inputs(inputs, mask_desc, mask_list, mw, mask_binary):
    x = np.asarray(inputs["x"], np.float32)
    wq = np.asarray(inputs["wq"], np.float32)
    wk = np.asarray(inputs["wk"], np.float32)
    wv = np.asarray(inputs["wv"], np.float32)
    wo = np.asarray(inputs["wo"], np.float32)
    attn_mask = np.asarray(inputs["attn_mask"], np.float32)
    start_pos = np.asarray(inputs["start_pos"], np.int32)

    bf = ml_dtypes.bfloat16
    KT = HID // 128

    inv_freq = 1.0 / (10000.0 ** (np.arange(0, HD, 2, dtype=np.float32) / HD))
    mt = attn_mask.T
    if mask_list:
        mask_arr = np.zeros((len(mask_list), 128, mw), np.float32)
        for i, (qb, kb) in enumerate(mask_list):
            d = mask_desc[qb][kb]
            if d[0] == "causal":
                off = d[1]
                mask_arr[i, :, 0:128] = mt[kb * 128:(kb + 1) * 128,
                                           qb * QB + off:qb * QB + off + 128]
            else:
                mask_arr[i, :, 0:QB] = mt[kb * 128:(kb + 1) * 128,
                                          qb * QB:(qb + 1) * QB]
    else:
        mask_arr = np.zeros((1, 128, mw), np.float32)
    if mask_binary:
        mask_arr = (mask_arr > NEG_THRESH).astype(ml_dtypes.bfloat16)

    # lhsT tile layout: [ct, p=hid_within_kt, kt, col_within_ct]
    def wtile2(w):
        c = w.shape[1]
        return np.ascontiguousarray(
            w.reshape(KT, 128, c // 128, 128).transpose(2, 1, 0, 3))

    in_maps = []
    for core in range(NCORES):
        g, r = divmod(core, TPG)
        xb = x[g * S:(g + 1) * S]                       # [S, HID]
        xt = np.ascontiguousarray(xb.T.reshape(KT, 128, S)).astype(bf)
        wq_c = (wq[:, r * HL * HD:(r + 1) * HL * HD] * SCALE)
        wk_c = wk[:, r * KVL * HD:(r + 1) * KVL * HD]
        wv_c = wv[:, r * KVL * HD:(r + 1) * KVL * HD]
        wo_c = wo[:, r * HIDL:(r + 1) * HIDL]           # [4096, 1024]

        pos = start_pos[g] + np.arange(S, dtype=np.float32)
        ang = pos[:, None] * inv_freq[None, :]          # [S, HD/2]
        cos = np.concatenate([np.cos(ang), np.cos(ang)], -1).T  # [HD, S]
        sin = np.concatenate([np.sin(ang), np.sin(ang)], -1).T

        in_maps.append({
            "xt": xt,
            "wq": wtile2(wq_c).astype(bf),
            "wk": wtile2(wk_c).astype(bf),
            "wv": wtile2(wv_c).astype(bf),
            "wo": np.ascontiguousarray(wo_c.reshape(KT, 128, HIDL)).astype(bf),
            "cos_t": np.ascontiguousarray(cos.astype(np.float32)),
            "sin_t": np.ascontiguousarray(sin.astype(np.float32)),
            "maskt": mask_arr,
        })
    return in_maps


def _make_runner(nc):
    """Cached jit over the bass module (adapted from
    concourse.bass2jax.run_bass_via_pjrt so repeat calls reuse one NEFF)."""
    import jax
    import jax.numpy as jnp
    from jax.sharding import Mesh, NamedSharding, PartitionSpec
    from jax.experimental.shard_map import shard_map

    import concourse.mybir as mybir
    from concourse import bass2jax

    bass2jax.install_neuronx_cc_hook()
    assert nc.dbg_addr is None
    partition_name = (nc.partition_id_tensor.name
                      if nc.partition_id_tensor else None)

    in_names, out_names, out_avals, out_shapes = [], [], [], []
    for alloc in nc.m.functions[0].allocations:
        if not isinstance(alloc, mybir.MemoryLocationSet):
            continue
        name = alloc.memorylocations[0].name
        if alloc.kind == "ExternalInput":
            if name != partition_name:
                in_names.append(name)
        elif alloc.kind == "ExternalOutput":
            assert alloc.tensor_shape is not None and alloc.dtype is not None
            shape = tuple(alloc.tensor_shape)
            npdt = mybir.dt.np(alloc.dtype)
            out_names.append(name)
            out_shapes.append((shape, npdt))
            out_avals.append(jax.core.ShapedArray(shape, npdt))

    n_params = len(in_names)
    n_outs = len(out_names)
    all_in_names = in_names + out_names
    if partition_name is not None:
        all_in_names = all_in_names + [partition_name]
    donate = tuple(range(n_params, n_params + n_outs))

    def _body(*args):
        operands = list(args)
        if partition_name is not None:
            operands.append(bass2jax.partition_id_tensor())
        outs = bass2jax._bass_exec_p.bind(
            *operands,
            out_avals=tuple(out_avals),
            in_names=tuple(all_in_names),
            out_names=tuple(out_names),
            lowering_input_output_aliases=(),
            sim_require_finite=True,
            sim_require_nnan=True,
            nc=nc,
        )
        return tuple(outs)

    devices = jax.devices()[:NCORES]
    mesh = Mesh(np.asarray(devices), ("core",))
    pc = PartitionSpec("core")
    sharded = jax.jit(
        shard_map(_body, mesh=mesh, in_specs=(pc,) * (n_params + n_outs),
                  out_specs=(pc,) * n_outs, check_rep=False),
        donate_argnums=donate, keep_unused=True)

    shard_dev = NamedSharding(mesh, pc)

    def make_zeros():
        return tuple(
            jax.device_put(np.zeros((NCORES * s[0], *s[1:]), d), shard_dev)
            for s, d in out_shapes)

    def put_inputs(in_maps):
        return [
            jax.device_put(
                np.concatenate([np.asarray(m[nm]) for m in in_maps], axis=0),
                shard_dev)
            for nm in in_names]

    def run_from_dev(in_dev, zeros):
        out_arrs = sharded(*in_dev, *zeros)
        jax.block_until_ready(out_arrs)
        return out_arrs

    def run(in_maps):
        out_arrs = run_from_dev(put_inputs(in_maps), make_zeros())
        return [
            {nm: np.asarray(out_arrs[i]).reshape(NCORES, *out_shapes[i][0])[c]
             for i, nm in enumerate(out_names)}
            for c in range(NCORES)]

    return {"run": run, "put_inputs": put_inputs, "make_zeros": make_zeros,
            "run_from_dev": run_from_dev, "sharded": sharded}


def _get_runner(mask_desc, mask_binary, repeat=1):
    key = (repr(mask_desc), mask_binary, repeat)
    cache = _STATE.setdefault("runners", {})
    if key not in cache:
        nc, mask_list, mw = _build_module(mask_desc, mask_binary, repeat)
        runner = _make_runner(nc)
        cache[key] = {"runner": runner, "mask_list": mask_list, "mw": mw,
                      "nc": nc}
    e = cache[key]
    _STATE.update({"run": e["runner"]["run"], "mask_list": e["mask_list"],
                   "mw": e["mw"], "nc": e["nc"], "runner": e["runner"]})
    return e["runner"]["run"], e["mask_list"], e["mw"]


def kernel(**inputs) -> np.ndarray:
    attn_mask = np.asarray(inputs["attn_mask"], np.float32)
    mask_desc = _classify_mask(attn_mask)
    mask_binary = _mask_is_binary(attn_mask)
    run, mask_list, mw = _get_runner(mask_desc, mask_binary)
    in_maps = _prep_core_inputs(inputs, mask_desc, mask_list, mw, mask_binary)
    results = run(in_maps)
    out = np.empty((B * S, HID), np.float32)
    for core in range(NCORES):
        g, r = divmod(core, TPG)
        out[g * S:(g + 1) * S, r * HIDL:(r + 1) * HIDL] = results[core]["outp"]
    return out
